# revision 7
# baseline (speedup 1.0000x reference)
"""Causal self-attention (B=1, S=4096, D=768, H=12) on 8 TRN2 NeuronCores.

Sharding: 4 head-groups (3 heads each) x 2 query-parity halves; no
collectives. Core c = 2*g + p handles heads [3g, 3g+3) and query rows
{r : r % 2 == p} (strided assignment balances causal work perfectly).

Per core:
  - K^T, V for its 3 heads over the full sequence (projected from x^T),
    Q^T for its strided query half (host supplies x^T[:, p::2]).
  - Flash-style causal attention with scores computed transposed
    ([k, q] layout) so the PV matmul needs no transposes; the softmax
    denominator comes from a ones-column appended to V; the causal
    "diagonal band" (1024 keys per 512-query tile, due to striding) is
    handled with a multiplicative {0,1} mask supplied by the host.
  - Partial output projection against its 192 rows of Wout.
Host sums the 4 head-group partials per parity, interleaves parities,
and adds bout.

All matmuls run in bf16 (f32 PSUM accumulation); softmax exp in f32.
"""
import os

import numpy as np
import ml_dtypes

import concourse.bass as bass
import concourse.mybir as mybir
import concourse.tile as tile
from concourse import bacc
from concourse.bass_utils import run_bass_kernel_spmd

BF16 = mybir.dt.bfloat16
F32 = mybir.dt.float32
NPBF16 = ml_dtypes.bfloat16

S = 4096          # sequence length
D = 768           # model dim
HD = 64           # head dim
HL = 3            # heads per core
DL = HL * HD      # 192 local qkv cols per core
SQ = S // 2       # 2048 local queries per core
NQT = 4           # q-tiles per core
QTW = 512         # q-tile width (local queries)
NKB = S // 128    # 32 key blocks of 128
NDC = D // 128    # 6 contraction chunks of 128 over D
GRP = 3           # score chunks per exp group (3 PSUM banks)
VW = HD + 1       # V' column stride per head (64 V cols + ones col)
SCALE = HD ** -0.5


def build_nc():
    nc = bacc.Bacc(None, target_bir_lowering=False)
    xT = nc.declare_dram_parameter("xT", [D, S], BF16, isOutput=False)
    xqT = nc.declare_dram_parameter("xqT", [D, SQ], BF16, isOutput=False)
    wk = nc.declare_dram_parameter("wk", [D, DL], BF16, isOutput=False)
    wq = nc.declare_dram_parameter("wq", [D, DL], BF16, isOutput=False)
    wv = nc.declare_dram_parameter("wv", [D, DL], BF16, isOutput=False)
    bk = nc.declare_dram_parameter("bk", [DL, 1], F32, isOutput=False)
    bq = nc.declare_dram_parameter("bq", [DL, 1], F32, isOutput=False)
    bv = nc.declare_dram_parameter("bv", [DL], F32, isOutput=False)
    wout = nc.declare_dram_parameter("wout", [DL, D], BF16, isOutput=False)
    maskT = nc.declare_dram_parameter("maskT", [1024, QTW], BF16, isOutput=False)
    out = nc.declare_dram_parameter("out", [SQ, D], F32, isOutput=True)

    from contextlib import ExitStack

    with tile.TileContext(nc) as tc, ExitStack() as ctx:
        # ---- persistent SBUF ----
        persist = ctx.enter_context(tc.tile_pool(name="persist", bufs=1))
        kT01 = persist.tile([128, S], BF16)         # K^T heads 0,1
        kT2 = persist.tile([64, S], BF16)           # K^T head 2
        qT01 = persist.tile([128, SQ], BF16)        # Q^T heads 0,1
        qT2 = persist.tile([64, SQ], BF16)
        aT01 = persist.tile([128, SQ], BF16)        # attn^T heads 0,1
        aT2 = persist.tile([64, SQ], BF16)
        vbig = persist.tile([128, NKB * HL * VW], BF16)  # V' blocks [k,195]
        mbig = persist.tile([128, 8, QTW], BF16)    # band masks
        bvb = persist.tile([128, DL], F32)          # bv broadcast over rows
        ones1 = persist.tile([1, 64], F32)
        bk0 = persist.tile([128, 1], F32)
        bk1 = persist.tile([64, 1], F32)
        bq0 = persist.tile([128, 1], F32)
        bq1 = persist.tile([64, 1], F32)

        nc.sync.dma_start(out=mbig, in_=maskT.rearrange("(b p) q -> p b q", p=128))
        nc.sync.dma_start(out=bvb, in_=bv[:].partition_broadcast(128))
        nc.sync.dma_start(out=bk0, in_=bk[0:128, :])
        nc.sync.dma_start(out=bk1, in_=bk[128:DL, :])
        nc.sync.dma_start(out=bq0, in_=bq[0:128, :])
        nc.sync.dma_start(out=bq1, in_=bq[128:DL, :])
        # ones columns of V' (memset whole tile; V data overwrites its cols)
        nc.vector.memset(vbig, 1.0)
        nc.vector.memset(ones1, 1.0)

        # ---- phase 1: projections ----
        with ExitStack() as pctx:
            xtp = pctx.enter_context(tc.tile_pool(name="xtp", bufs=1))
            wp = pctx.enter_context(tc.tile_pool(name="wp", bufs=1))
            pp = pctx.enter_context(tc.tile_pool(name="pp", bufs=2, space="PSUM"))
            pvp = pctx.enter_context(tc.tile_pool(name="pvp", bufs=2, space="PSUM"))

            xt = []
            for i in range(NDC):
                t = xtp.tile([128, S], BF16, name=f"xt{i}")
                nc.sync.dma_start(out=t, in_=xT[i * 128:(i + 1) * 128, :])
                xt.append(t)
            xq = []
            for i in range(NDC):
                t = xtp.tile([128, SQ], BF16, name=f"xq{i}")
                nc.sync.dma_start(out=t, in_=xqT[i * 128:(i + 1) * 128, :])
                xq.append(t)
            wk_sb, wq_sb, wv_sb = [], [], []
            for nm, dram, lst in (("wk", wk, wk_sb), ("wq", wq, wq_sb),
                                  ("wv", wv, wv_sb)):
                for i in range(NDC):
                    t = wp.tile([128, DL], BF16, name=f"{nm}{i}")
                    nc.sync.dma_start(out=t, in_=dram[i * 128:(i + 1) * 128, :])
                    lst.append(t)

            # K^T and Q^T: out[m, n] = sum_d W[d, m] * xT[d, n]
            for dst01, dst2, w_sb, rhs_all, nfull, b0, b1 in (
                (kT01, kT2, wk_sb, xt, S, bk0, bk1),
                (qT01, qT2, wq_sb, xq, SQ, bq0, bq1),
            ):
                for m in range(2):          # m0: cols 0:128 (h0,h1), m1: 128:192 (h2)
                    mw = 128 if m == 0 else 64
                    msl = slice(0, 128) if m == 0 else slice(128, DL)
                    for n in range(nfull // 512):
                        nsl = slice(n * 512, (n + 1) * 512)
                        ps = pp.tile([128, 512], F32, name="pk", tag="pk")
                        for kc in range(NDC):
                            nc.tensor.matmul(
                                ps[:mw, :], lhsT=w_sb[kc][:, msl],
                                rhs=rhs_all[kc][:, nsl],
                                start=(kc == 0), stop=(kc == NDC - 1),
                            )
                        dst = dst01 if m == 0 else dst2
                        bias = (b0 if m == 0 else b1)
                        nc.vector.tensor_scalar_add(
                            out=dst[0:mw, nsl], in0=ps[:mw, :], scalar1=bias[:mw, :])

            # V (natural [k, d] layout) + ones cols, into vbig
            for kb in range(NKB):
                pv = pvp.tile([128, DL], F32, name="pv", tag="pv")
                ksl = slice(kb * 128, (kb + 1) * 128)
                for kc in range(NDC):
                    nc.tensor.matmul(
                        pv, lhsT=xt[kc][:, ksl], rhs=wv_sb[kc],
                        start=(kc == 0), stop=(kc == NDC - 1),
                    )
                for h in range(HL):
                    voff = kb * HL * VW + h * VW
                    nc.vector.tensor_add(
                        out=vbig[:, voff:voff + HD],
                        in0=pv[:, h * HD:(h + 1) * HD],
                        in1=bvb[:, h * HD:(h + 1) * HD],
                    )

        # ---- phase 2: attention ----
        with ExitStack() as actx:
            psp = actx.enter_context(tc.tile_pool(name="psp", bufs=2, space="PSUM"))
            pop = actx.enter_context(tc.tile_pool(name="pop", bufs=1, space="PSUM"))
            ep = actx.enter_context(tc.tile_pool(name="ep", bufs=3))
            emp = actx.enter_context(tc.tile_pool(name="emp", bufs=4))
            rp = actx.enter_context(tc.tile_pool(name="rp", bufs=2))

            for h in range(HL):
                if h == 0:
                    kT_h, qT_h = kT01[0:64], qT01[0:64]
                    aT_h = aT01[0:64]
                elif h == 1:
                    kT_h, qT_h = kT01[64:128], qT01[64:128]
                    aT_h = aT01[64:128]
                else:
                    kT_h, qT_h = kT2[0:64], qT2[0:64]
                    aT_h = aT2[0:64]
                for t in range(NQT):
                    qsl = slice(t * QTW, (t + 1) * QTW)
                    nkb = 8 * (t + 1)
                    po = pop.tile([VW, 512], F32, name="po", tag="po")
                    for kb0 in range(0, nkb, GRP):
                        g = min(GRP, nkb - kb0)
                        ps = psp.tile([128, GRP * 512], F32, name="ps", tag="ps")
                        for gi in range(g):
                            kb = kb0 + gi
                            nc.tensor.matmul(
                                ps[:, gi * 512:(gi + 1) * 512],
                                lhsT=kT_h[:, kb * 128:(kb + 1) * 128],
                                rhs=qT_h[:, qsl],
                                start=True, stop=True,
                            )
                        eT = ep.tile([128, GRP * 512], BF16, name="eT", tag="eT")
                        nc.scalar.activation(
                            out=eT[:, :g * 512], in_=ps[:, :g * 512],
                            func=mybir.ActivationFunctionType.Exp, scale=SCALE)
                        for gi in range(g):
                            kb = kb0 + gi
                            src = eT[:, gi * 512:(gi + 1) * 512]
                            if kb >= 8 * t:          # diagonal band: mask
                                b = kb - 8 * t
                                em = emp.tile([128, 512], BF16, name="em", tag="em")
                                nc.vector.tensor_mul(
                                    out=em, in0=src, in1=mbig[:, b, :])
                                src = em
                            voff = kb * HL * VW + h * VW
                            nc.tensor.matmul(
                                po[0:VW, :], lhsT=vbig[:, voff:voff + VW],
                                rhs=src,
                                start=(kb == 0), stop=(kb == nkb - 1),
                                skip_group_check=True,
                            )
                    rec = rp.tile([1, 512], F32, name="rec", tag="rec")
                    nc.vector.reciprocal(out=rec, in_=po[HD:VW, :])
                    pb = psp.tile([64, 512], F32, name="pb", tag="pb", bufs=1)
                    nc.tensor.matmul(pb, lhsT=ones1, rhs=rec,
                                     start=True, stop=True)
                    recb = rp.tile([64, 512], F32, name="recb", tag="recb")
                    nc.vector.tensor_copy(out=recb, in_=pb)
                    nc.vector.tensor_mul(
                        out=aT_h[:, qsl], in0=po[0:HD, :], in1=recb)

        # ---- phase 3: output projection ----
        with ExitStack() as octx:
            wop = octx.enter_context(tc.tile_pool(name="wop", bufs=1))
            pq = octx.enter_context(tc.tile_pool(name="pq", bufs=2, space="PSUM"))
            osb = octx.enter_context(tc.tile_pool(name="osb", bufs=3))

            wo0 = wop.tile([128, D], BF16)
            wo1 = wop.tile([64, D], BF16)
            nc.sync.dma_start(out=wo0, in_=wout[0:128, :])
            nc.sync.dma_start(out=wo1, in_=wout[128:DL, :])

            for qt in range(SQ // 128):
                qsl = slice(qt * 128, (qt + 1) * 128)
                pot = pq.tile([128, D], F32, name="pot", tag="pot")
                for ncol in range(2):
                    csl = slice(ncol * 512, min((ncol + 1) * 512, D))
                    nc.tensor.matmul(
                        pot[:, csl], lhsT=aT01[:, qsl], rhs=wo0[:, csl],
                        start=True, stop=False, skip_group_check=True)
                    nc.tensor.matmul(
                        pot[:, csl], lhsT=aT2[:, qsl], rhs=wo1[:, csl],
                        start=False, stop=True, skip_group_check=True)
                ot = osb.tile([128, D], F32, name="ot", tag="ot")
                nc.vector.tensor_copy(out=ot, in_=pot)
                nc.sync.dma_start(out=out[qsl, :], in_=ot)

    nc.finalize()
    return nc


_NC_CACHE = {}


def _get_nc():
    if "nc" not in _NC_CACHE:
        _NC_CACHE["nc"] = build_nc()
    return _NC_CACHE["nc"]


def kernel(x, Wqkv, bqkv, Wout, bout):
    B, S_, D_ = x.shape
    assert (B, S_, D_) == (1, S, D)
    nc = _get_nc()

    xT_np = np.ascontiguousarray(x[0].T).astype(NPBF16)          # [768, 4096]
    in_maps = []
    for c in range(8):
        g, p = c // 2, c % 2
        csl = slice(DL * g, DL * (g + 1))
        kk = np.arange(1024, dtype=np.int64)[:, None]
        jj = np.arange(QTW, dtype=np.int64)[None, :]
        mask = (kk <= 2 * jj + p).astype(NPBF16)
        in_maps.append({
            "xT": xT_np,
            "xqT": np.ascontiguousarray(xT_np[:, p::2]),
            "wk": np.ascontiguousarray(Wqkv[:, D + DL * g:D + DL * (g + 1)]).astype(NPBF16),
            "wq": np.ascontiguousarray(Wqkv[:, csl]).astype(NPBF16),
            "wv": np.ascontiguousarray(Wqkv[:, 2 * D + DL * g:2 * D + DL * (g + 1)]).astype(NPBF16),
            "bk": np.ascontiguousarray(bqkv[D + DL * g:D + DL * (g + 1)]).astype(np.float32).reshape(DL, 1),
            "bq": np.ascontiguousarray(bqkv[csl]).astype(np.float32).reshape(DL, 1),
            "bv": np.ascontiguousarray(bqkv[2 * D + DL * g:2 * D + DL * (g + 1)]).astype(np.float32),
            "wout": np.ascontiguousarray(Wout[csl, :]).astype(NPBF16),
            "maskT": mask,
        })

    trace = bool(int(os.environ.get("ATTN_TRACE", "0")))
    res = run_bass_kernel_spmd(nc, in_maps, core_ids=list(range(8)), trace=trace)
    if trace:
        _NC_CACHE["last_result"] = res

    out_full = np.zeros((S, D), np.float32)
    for p in range(2):
        acc = np.zeros((SQ, D), np.float32)
        for g in range(4):
            acc += res.results[2 * g + p]["out"]
        out_full[p::2] = acc
    out_full += bout.astype(np.float32)[None, :]
    return out_full[None].astype(np.float32)
